# revision 1
# baseline (speedup 1.0000x reference)
"""BitNet attention (D_MODEL=2048, 16 heads, B=2, T=2048) on 8 TRN2 cores.

Sharding: tensor-parallel over heads — each core owns 2 heads (256 dims) of
the q/k/v projections (column-parallel) and 256 output columns of out_proj
(column-parallel on a full, AllGather-ed quantized activation).

Numerics:
- act/weight quantization produce integer-grid values held EXACTLY in bf16;
  all four projection matmuls run as exact integer bf16 matmuls (fp32 PSUM),
  descaled afterwards.
- RNE rounding via the +/-1.5*2^23 magic-number trick (matches jnp.round).
- attention (scores, exp(s)@V, denominators) runs in float32r (FP22) at full
  PE rate; softmax max-subtraction is skipped (scores are O(1), exp is safe,
  result is mathematically identical).
- softmax denominators via ones-column matmuls, with normalization fused into
  the PSUM->SBUF eviction of the attention-output transpose.

Cross-core collectives: AllReduce(add) for weight-quant means, AllReduce(max)
for the out_proj per-token absmax, AllGather for the quantized attn output.
"""

import numpy as np

import concourse.bass as bass
import concourse.mybir as mybir
import concourse.tile as tile
from concourse.bass_utils import run_bass_kernel_spmd
from concourse.vector_clock import ScopedClock

DT = mybir.dt
ALU = mybir.AluOpType
ACTF = mybir.ActivationFunctionType

N_CORES = 8
P = 128
FD = 2048          # d_model
B, T = 2, 2048
BT = B * T
OC = FD // N_CORES  # 256: per-core head dims / out_proj cols
NF = FD // P        # 16 feature tiles
NT = T // P         # 16 token tiles per batch
TB = 512            # t1 block
NTB = T // TB       # 4
MAGIC = 12582912.0  # 1.5 * 2**23
RG = [list(range(N_CORES))]

# ---------------------------------------------------------------------------
# Workaround: the bundled walrus rejects >1 sem-wait on a Drain (CTRL_NO_STRUCT)
# instruction. Split the TileContext tail drain into single-wait drains.
_orig_drain_and_barrier = tile.TileContext._drain_and_barrier


def _patched_drain_and_barrier(self, tick_clock, wait_clock):
    nc = self.nc
    drain_inst = nc.sync.drain()
    wait_clock.add_sem_waits(
        drain_inst.ins, ScopedClock({None: tick_clock.global_clock})
    )
    si = drain_inst.ins.sync_info
    waits = list(si.on_wait or []) if si is not None else []
    if len(waits) > 1:
        si.on_wait = waits[:1]
        for w in waits[1:]:
            extra = nc.sync.drain()
            extra.ins.sync_info = mybir.SyncInfo(on_wait=[w], on_update=[])

    nc.all_engine_barrier()
    assert self.sems is not None
    popped = nc._tile_sem_poison_stack.pop()
    assert popped is self._sem_poison
    nc.clear_and_free_semaphores(list(self.sems.allocated().values()))
    nc.all_engine_barrier()


def _install_patch():
    tile.TileContext._drain_and_barrier = _patched_drain_and_barrier


# The bundled walrus also rejects more than MAX_WAITS sem-waits on regular
# instructions. Peel extra waits onto same-engine NoOps inserted just before.
def _split_excess_waits(nc, max_waits):
    n_new = 0
    for fn in nc.m.functions:
        for blk in fn.blocks:
            il = blk.instructions
            out = []
            for inst in il:
                si = getattr(inst, "sync_info", None)
                waits = list(si.on_wait) if (si is not None and si.on_wait) else []
                if len(waits) > max_waits:
                    extra = waits[:-max_waits] if max_waits else waits
                    keep = waits[-max_waits:] if max_waits else []
                    step = max(1, max_waits)
                    for k in range(0, len(extra), step):
                        n_new += 1
                        nop = mybir.InstNoOp(
                            name=f"WSP{n_new}",
                            sync_info=mybir.SyncInfo(
                                on_wait=extra[k:k + step], on_update=[]),
                            bass_nofuse=True,
                            engine=inst.engine,
                        )
                        nc.register_instruction(nop, overwrite=True)
                        out.append(nop)
                    si.on_wait = keep
                out.append(inst)
            il[:] = out
    return n_new


# ---------------------------------------------------------------------------


def _f32r(ap):
    return ap.bitcast(DT.float32r)


def build_kernel():
    import os
    _install_patch()
    nc = bass.Bass("TRN2", target_bir_lowering=False, debug=False,
                   num_devices=N_CORES)
    x_in = nc.dram_tensor("x", [BT, FD], DT.float32, kind="ExternalInput")
    wT = {
        w: nc.dram_tensor(f"w{w}T", [FD, OC], DT.float32, kind="ExternalInput")
        for w in "qkvo"
    }
    y_out = nc.dram_tensor("y", [B, T, OC], DT.float32, kind="ExternalOutput")
    dbg = None
    if os.environ.get("KDBG"):
        dbg = {
            "R": nc.dram_tensor("dbg_R", [P, T], DT.float32, kind="ExternalOutput"),
            "QT0": nc.dram_tensor("dbg_QT0", [P, T], DT.float32, kind="ExternalOutput"),
            "V0": nc.dram_tensor("dbg_V0", [P, OC], DT.float32, kind="ExternalOutput"),
            "PVT0": nc.dram_tensor("dbg_PVT0", [P, T], DT.float32, kind="ExternalOutput"),
            "den0": nc.dram_tensor("dbg_den0", [1, T], DT.float32, kind="ExternalOutput"),
            "denT0": nc.dram_tensor("dbg_denT0", [P, NT], DT.float32, kind="ExternalOutput"),
            "sqa": nc.dram_tensor("dbg_sqa", [P, NT], DT.float32, kind="ExternalOutput"),
            "qaT0": nc.dram_tensor("dbg_qaT0", [P, T], DT.bfloat16, kind="ExternalOutput"),
            "qaTg": nc.dram_tensor("dbg_qaTg", [FD, T], DT.bfloat16, kind="ExternalOutput"),
        }

    with tile.TileContext(nc) as tc:
        for _rep in range(int(os.environ.get("KREPEAT", "1"))):
            _body(nc, tc, x_in, wT, y_out,
                  int(os.environ.get("KCUT", "99")),
                  dbg if _rep == 0 else None)
    import os
    _split_excess_waits(nc, int(os.environ.get("BASS_MAX_WAITS", "1")))
    return nc


def _body(nc, tc, x_in, wT, y_out, kcut=99, dbg=None):
    fp32 = DT.float32
    bf16 = DT.bfloat16

    from contextlib import ExitStack
    stack = ExitStack()
    const = stack.enter_context(tc.tile_pool(name="const", bufs=1))
    dram = stack.enter_context(tc.tile_pool(name="dram", bufs=1, space="DRAM"))

    ones_col = const.tile([P, 1], fp32, tag="ones_col", name="ones_col")
    nc.gpsimd.memset(ones_col[:], 1.0)
    ones_row = const.tile([1, P], fp32, tag="ones_row", name="ones_row")
    nc.gpsimd.memset(ones_row[:], 1.0)
    ones_col_r = const.tile([P, 1], DT.float32r, tag="ones_col_r", name="ones_col_r")
    nc.vector.tensor_copy(ones_col_r[:], ones_col[:])
    ident = const.tile([P, P], fp32, tag="ident", name="ident")
    from concourse.masks import make_identity
    make_identity(nc, ident[:])

    # quantized weights (persistent): 4 x 16 tiles [128, 256] bf16
    wqbf = {
        w: [const.tile([P, OC], bf16, tag=f"w{w}bf{i}", name=f"w{w}bf{i}") for i in range(NF)]
        for w in "qkvo"
    }
    swb = const.tile([P, 8], fp32, tag="swb", name="swb")   # cols 0-3: s_w(q,k,v,o); 4-7: 1/s_w
    magicv = const.tile([P, 1], fp32, tag="magicv", name="magicv")
    nc.gpsimd.memset(magicv[:], MAGIC)
    cvec = const.tile([P, 1], fp32, tag="cvec", name="cvec")  # 1/(s_wq*s_wk*sqrt(128))

    # ---------------- P0: weight quantization ----------------
    with (
        tc.tile_pool(name="wstage", bufs=3) as wstage,
        tc.tile_pool(name="p0", bufs=1) as p0,
        tc.tile_pool(name="p0ps", bufs=2, space="PSUM") as p0ps,
    ):
        asum = p0.tile([P, 4 * NF], fp32, tag="asum", name="asum")
        tots = p0.tile([P, 4], fp32, tag="tots", name="tots")
        for wi, w in enumerate("qkvo"):
            for i in range(NF):
                st = wstage.tile([P, OC], fp32, tag="wst", name="wst")
                nc.sync.dma_start(out=st[:], in_=wT[w][i * P:(i + 1) * P, :])
                nc.vector.tensor_reduce(
                    asum[:, wi * NF + i: wi * NF + i + 1], st[:],
                    mybir.AxisListType.X, ALU.add, apply_absolute_value=True,
                )
            nc.vector.tensor_reduce(
                tots[:, wi:wi + 1], asum[:, wi * NF:(wi + 1) * NF],
                mybir.AxisListType.X, ALU.add,
            )
        ps14 = p0ps.tile([1, 4], fp32, tag="ps14", name="ps14")
        nc.tensor.matmul(ps14[:], lhsT=ones_col[:], rhs=tots[:],
                         start=True, stop=True)
        sums4 = p0.tile([1, 4], fp32, tag="sums4", name="sums4")
        nc.vector.tensor_copy(sums4[:], ps14[:])
        cc_in = dram.tile([1, 4], fp32, tag="cc_in", name="cc_in")
        cc_out = dram.tile([1, 4], fp32, tag="cc_out", name="cc_out")
        nc.sync.dma_start(out=cc_in[:], in_=sums4[:])
        nc.gpsimd.collective_compute(
            "AllReduce", ALU.add, replica_groups=RG,
            ins=[cc_in.opt()], outs=[cc_out.opt()],
        )
        row8 = p0.tile([1, 8], fp32, tag="row8", name="row8")
        gs = p0.tile([1, 4], fp32, tag="gs", name="gs")
        nc.sync.dma_start(out=gs[:], in_=cc_out[:])
        # minv = max(mean, 1e-5) = 1/s_w ; s_w = 1/minv
        nc.vector.tensor_scalar(row8[:, 4:8], gs[:], 1.0 / (FD * FD), 1e-5,
                                ALU.mult, ALU.max)
        nc.vector.reciprocal(row8[:, 0:4], row8[:, 4:8])
        psb = p0ps.tile([P, 8], fp32, tag="psb", name="psb")
        nc.tensor.matmul(psb[:], lhsT=ones_row[:], rhs=row8[:],
                         start=True, stop=True)
        nc.vector.tensor_copy(swb[:], psb[:])
        cv1 = p0.tile([P, 1], fp32, tag="cv1", name="cv1")
        nc.vector.tensor_tensor(cv1[:], swb[:, 4:5], swb[:, 5:6], ALU.mult)
        nc.vector.tensor_scalar_mul(cvec[:], cv1[:], float(P) ** -0.5)

        # quantize: round(w * s_w) clipped to [-1, 1], bf16
        for wi, w in enumerate("qkvo"):
            for i in range(NF):
                st = wstage.tile([P, OC], fp32, tag="wst", name="wst")
                nc.sync.dma_start(out=st[:], in_=wT[w][i * P:(i + 1) * P, :])
                t1 = wstage.tile([P, OC], fp32, tag="wt1", name="wt1")
                nc.scalar.activation(t1[:], st[:], ACTF.Identity,
                                     scale=swb[:, wi:wi + 1], bias=magicv[:])
                t2 = wstage.tile([P, OC], fp32, tag="wt2", name="wt2")
                nc.vector.tensor_scalar(t2[:], t1[:], -MAGIC, 1.0,
                                        ALU.add, ALU.min)
                nc.vector.tensor_scalar_max(wqbf[w][i][:], t2[:], -1.0)

    # ---------------- per-batch pipeline ----------------
    if kcut >= 1:
        for b in range(B if kcut >= 99 else 1):
            _batch(nc, tc, x_in, y_out, b, wqbf, swb, cvec, ones_col_r,
                   ones_row, ident, dram, magicv, kcut, dbg if b == 0 else None)

    stack.close()


def _batch(nc, tc, x_in, y_out, b, wqbf, swb, cvec, ones_col_r, ones_row,
           ident, dram, magicv, kcut=99, dbg=None):
    fp32 = DT.float32
    bf16 = DT.bfloat16
    X = mybir.AxisListType.X

    from contextlib import ExitStack
    with ExitStack() as bstack:
        # lifetime A: P1 write -> P3 read
        pA = bstack.enter_context(tc.tile_pool(name=f"A{b}", bufs=1))
        QT = [pA.tile([P, T], DT.float32r, tag=f"QT{h}", name=f"QT{h}") for h in range(2)]
        KT = [pA.tile([P, T], DT.float32r, tag=f"KT{h}", name=f"KT{h}") for h in range(2)]
        V = [pA.tile([P, OC], DT.float32r, tag=f"V{j}", name=f"V{j}") for j in range(NT)]

        # ---------------- P1 + P2 ----------------
        with (
            tc.tile_pool(name=f"A2_{b}", bufs=1) as pA2,
            tc.tile_pool(name=f"xstage{b}", bufs=2) as xstage,
            tc.tile_pool(name=f"p1s{b}", bufs=4) as p1s,
            tc.tile_pool(name=f"qb{b}", bufs=2) as qbp,
        ):
            sinv = pA2.tile([P, NT], fp32, tag="sinv", name="sinv")
            R = pA2.tile([P, T], fp32, tag="R", name="R")
            qxT = [pA2.tile([P, T], bf16, tag=f"qxT{i}", name=f"qxT{i}")
                   for i in range(NF)]
            d_qx = dram.tile([T, FD], bf16, tag=f"d_qx{b}", name=f"d_qx{b}")
            p1ctx = tc.tile_pool(name=f"tqps{b}", bufs=2, space="PSUM")
            tqps = p1ctx.__enter__()
            for j in range(NT):
                xt = xstage.tile([P, FD], fp32, tag="xt", name="xt")
                nc.sync.dma_start(
                    out=xt[:], in_=x_in[b * T + j * P: b * T + (j + 1) * P, :])
                am = p1s.tile([P, 1], fp32, tag="am", name="am")
                nc.vector.tensor_reduce(am[:], xt[:], X, ALU.max,
                                        apply_absolute_value=True)
                amc = p1s.tile([P, 1], fp32, tag="amc", name="amc")
                nc.vector.tensor_scalar_max(amc[:], am[:], 1e-5)
                sv = p1s.tile([P, 1], fp32, tag="sv", name="sv")
                nc.vector.reciprocal(sv[:], amc[:])
                svec = p1s.tile([P, 1], fp32, tag="svec", name="svec")
                nc.vector.tensor_scalar_mul(svec[:], sv[:], 127.0)
                nc.vector.tensor_scalar_mul(sinv[:, j:j + 1], amc[:],
                                            1.0 / 127.0)
                qb = qbp.tile([P, FD], bf16, tag="qb", name="qb")
                for h in range(2):
                    hs = slice(h * (FD // 2), (h + 1) * (FD // 2))
                    tq = tqps.tile([P, FD // 2], fp32, tag="tq", name="tq")
                    nc.scalar.activation(tq[:], xt[:, hs], ACTF.Identity,
                                         scale=svec[:], bias=magicv[:])
                    nc.vector.tensor_scalar_add(qb[:, hs], tq[:], -MAGIC)
                nc.sync.dma_start(out=d_qx[j * P:(j + 1) * P, :], in_=qb[:])
                for i in range(NF):
                    nc.sync.dma_start(
                        out=qxT[i][:, j * P:(j + 1) * P],
                        in_=d_qx[j * P:(j + 1) * P, i * P:(i + 1) * P],
                        transpose=True,
                    )

            p1ctx.__exit__(None, None, None)

            # R = broadcast of 1/s per token: PE-transpose sinv to a
            # t-ordered [NT, P] layout, DRAM-roundtrip it to a [1, T] row,
            # then broadcast across partitions with a ones matmul.
            with tc.tile_pool(name=f"bc{b}", bufs=2, space="PSUM") as bcps:
                pst0 = bcps.tile([NT, P], fp32, tag="sT", name="pst0")
                nc.tensor.transpose(pst0[:], sinv[:], ident[:])
                sinvT = p1s.tile([NT, P], fp32, tag="sinvT", name="sinvT", bufs=1)
                nc.vector.tensor_copy(sinvT[:], pst0[:])
                d_sinv = dram.tile([NT, P], fp32, tag=f"d_sinv{b}",
                                   name=f"d_sinv{b}")
                nc.sync.dma_start(out=d_sinv[:], in_=sinvT[:])
                srow = p1s.tile([1, T], fp32, tag="srow", name="srow", bufs=1)
                nc.sync.dma_start(
                    out=srow[:],
                    in_=d_sinv.rearrange("j p -> (j p)").unsqueeze(0))
                for q4 in range(NTB):
                    q4s = slice(q4 * TB, (q4 + 1) * TB)
                    psb2 = bcps.tile([P, TB], fp32, tag="bc", name="psb2")
                    nc.tensor.matmul(psb2[:], lhsT=ones_row[:],
                                     rhs=srow[:, q4s], start=True, stop=True)
                    nc.vector.tensor_copy(R[:, q4s], psb2[:])

            if dbg:
                nc.sync.dma_start(out=dbg["R"][:], in_=R[:])
            # ---------------- P2: projections ----------------
            if kcut < 2:
                return
            with (
                tc.tile_pool(name=f"qkps{b}", bufs=3, space="PSUM") as qkps,
                tc.tile_pool(name=f"vps{b}", bufs=2, space="PSUM") as vps,
            ):
                for wname, dst in (("q", QT), ("k", KT)):
                    for o in range(2):
                        for t4 in range(NTB):
                            ps = qkps.tile([P, TB], fp32, tag="qk", name="qk")
                            for i in range(NF):
                                nc.tensor.matmul(
                                    ps[:],
                                    lhsT=wqbf[wname][i][:, o * P:(o + 1) * P],
                                    rhs=qxT[i][:, t4 * TB:(t4 + 1) * TB],
                                    start=(i == 0), stop=(i == NF - 1),
                                )
                            nc.vector.tensor_tensor(
                                dst[o][:, t4 * TB:(t4 + 1) * TB], ps[:],
                                R[:, t4 * TB:(t4 + 1) * TB], ALU.mult,
                            )
                for j in range(NT):
                    ps = vps.tile([P, OC], fp32, tag="v", name="v")
                    for i in range(NF):
                        nc.tensor.matmul(
                            ps[:],
                            lhsT=qxT[i][:, j * P:(j + 1) * P],
                            rhs=wqbf["v"][i][:],
                            start=(i == 0), stop=(i == NF - 1),
                        )
                    nc.vector.tensor_scalar(V[j][:], ps[:], sinv[:, j:j + 1],
                                            swb[:, 6:7], ALU.mult, ALU.mult)

        if kcut < 3:
            return
        if dbg:
            nc.sync.dma_start(out=dbg["QT0"][:].bitcast(DT.float32r), in_=QT[0][:])
            nc.sync.dma_start(out=dbg["V0"][:].bitcast(DT.float32r), in_=V[0][:])
        # lifetime B: P3 write -> P4/P5 read
        pB = bstack.enter_context(tc.tile_pool(name=f"B{b}", bufs=1))
        PVT = [pB.tile([P, T], fp32, tag=f"PVT{h}", name=f"PVT{h}")
               for h in range(2)]
        den = [pB.tile([1, T], fp32, tag=f"den{h}", name=f"den{h}")
               for h in range(2)]
        Ry = pB.tile([P, NT], fp32, tag="Ry", name="Ry")

        # ---------------- P3: attention ----------------
        with (
            tc.tile_pool(name=f"sps{b}", bufs=3, space="PSUM") as sps,
            tc.tile_pool(name=f"pvps{b}", bufs=2, space="PSUM") as pvps,
            tc.tile_pool(name=f"dnps{b}", bufs=2, space="PSUM") as dnps,
            tc.tile_pool(name=f"expp{b}", bufs=18) as expp,
        ):
            for hl in range(2):
                for t1b in range(NTB):
                    t1s = slice(t1b * TB, (t1b + 1) * TB)
                    pv = pvps.tile([P, TB], fp32, tag="pv", name="pv")
                    dn = dnps.tile([1, TB], fp32, tag="dn", name="dn")
                    for j in range(NT):
                        ss = sps.tile([P, TB], fp32, tag="ss", name="ss")
                        nc.tensor.matmul(
                            ss[:],
                            lhsT=KT[hl][:, j * P:(j + 1) * P],
                            rhs=QT[hl][:, t1s],
                            start=True, stop=True,
                        )
                        ex = expp.tile([P, TB], DT.float32r, tag="ex", name="ex")
                        nc.scalar.activation(ex[:], ss[:], ACTF.Exp,
                                             scale=cvec[:])
                        nc.tensor.matmul(
                            pv[:],
                            lhsT=V[j][:, hl * P:(hl + 1) * P],
                            rhs=ex[:],
                            start=(j == 0), stop=(j == NT - 1),
                        )
                        nc.tensor.matmul(
                            dn[:],
                            lhsT=ones_col_r[:],
                            rhs=ex[:],
                            start=(j == 0), stop=(j == NT - 1),
                        )
                    nc.vector.tensor_copy(PVT[hl][:, t1s], pv[:])
                    nc.vector.tensor_copy(den[hl][:, t1s], dn[:])

        if dbg:
            nc.sync.dma_start(out=dbg["PVT0"][:], in_=PVT[0][:])
            nc.sync.dma_start(out=dbg["den0"][:], in_=den[0][:])
        if kcut < 4:
            return
        # ---------------- P4: normalize, quantize, gather ----------------
        qaT = [None, None]
        with (
            tc.tile_pool(name=f"p4{b}", bufs=1) as p4,
            tc.tile_pool(name=f"p4s{b}", bufs=2) as p4s,
            tc.tile_pool(name=f"tps{b}", bufs=3, space="PSUM") as tps,
            tc.tile_pool(name=f"tpd{b}", bufs=2, space="PSUM") as tpd,
        ):
            PVn = [p4.tile([P, OC], fp32, tag=f"PVn{j}", name=f"PVn{j}")
                   for j in range(NT)]
            rmax = p4.tile([P, NT], fp32, tag="rmax", name="rmax")
            sqa = p4.tile([P, NT], fp32, tag="sqa", name="sqa")
            qaTl = [p4.tile([P, T], bf16, tag=f"qaT{d}", name=f"qaT{d}")
                    for d in range(2)]
            denT = [p4.tile([P, NT], fp32, tag=f"denT{h}", name=f"denT{h}")
                    for h in range(2)]
            for hl in range(2):
                rd = p4.tile([1, T], fp32, tag=f"rd{hl}", name=f"rd{hl}")
                nc.vector.reciprocal(rd[:], den[hl][:])
                d_den = dram.tile([1, T], fp32, tag=f"d_den{b}{hl}",
                                  name=f"d_den{b}{hl}")
                nc.sync.dma_start(out=d_den[:], in_=rd[:])
                den16 = p4.tile([NT, P], fp32, tag=f"den16_{hl}",
                                name=f"den16_{hl}")
                nc.sync.dma_start(
                    out=den16[:],
                    in_=d_den.rearrange("o (j p) -> (o j) p", p=P))
                pstd = tpd.tile([P, NT], fp32, tag="dT", name="pstd")
                nc.tensor.transpose(pstd[:], den16[:], ident[0:NT, 0:NT])
                nc.vector.tensor_copy(denT[hl][:], pstd[:])
            for hl in range(2):
                for j in range(NT):
                    pst = tps.tile([P, P], fp32, tag="t", name="t")
                    nc.tensor.transpose(pst[:], PVT[hl][:, j * P:(j + 1) * P],
                                        ident[:])
                    nc.vector.tensor_scalar(
                        PVn[j][:, hl * P:(hl + 1) * P], pst[:],
                        denT[hl][:, j:j + 1], None, ALU.mult,
                    )
            for j in range(NT):
                nc.vector.tensor_reduce(rmax[:, j:j + 1], PVn[j][:], X,
                                        ALU.max, apply_absolute_value=True)
            d_rm_in = dram.tile([P, NT], fp32, tag=f"d_rm_in{b}",
                                name=f"d_rm_in{b}")
            d_rm_out = dram.tile([P, NT], fp32, tag=f"d_rm_out{b}",
                                 name=f"d_rm_out{b}")
            nc.sync.dma_start(out=d_rm_in[:], in_=rmax[:])
            nc.gpsimd.collective_compute(
                "AllReduce", ALU.max, replica_groups=RG,
                ins=[d_rm_in.opt()], outs=[d_rm_out.opt()],
            )
            rmg = p4.tile([P, NT], fp32, tag="rmg", name="rmg")
            nc.sync.dma_start(out=rmg[:], in_=d_rm_out[:])
            mxt = p4.tile([P, NT], fp32, tag="mxt", name="mxt")
            nc.vector.tensor_scalar_max(mxt[:], rmg[:], 1e-5)
            rc = p4.tile([P, NT], fp32, tag="rc", name="rc")
            nc.vector.reciprocal(rc[:], mxt[:])
            nc.vector.tensor_scalar_mul(sqa[:], rc[:], 127.0)
            nc.vector.tensor_scalar(Ry[:], mxt[:], swb[:, 7:8], 1.0 / 127.0,
                                    ALU.mult, ALU.mult)
            if dbg:
                nc.sync.dma_start(out=dbg["denT0"][:], in_=denT[0][:])
                nc.sync.dma_start(out=dbg["sqa"][:], in_=sqa[:])
            d_qa = dram.tile([T, OC], bf16, tag=f"d_qa{b}", name=f"d_qa{b}")
            for j in range(NT):
                tq = p4s.tile([P, OC], fp32, tag="tq4", name="tq4")
                nc.scalar.activation(tq[:], PVn[j][:], ACTF.Identity,
                                     scale=sqa[:, j:j + 1], bias=magicv[:])
                qa = p4s.tile([P, OC], bf16, tag="qa", name="qa")
                nc.vector.tensor_scalar_add(qa[:], tq[:], -MAGIC)
                nc.sync.dma_start(out=d_qa[j * P:(j + 1) * P, :], in_=qa[:])
                for dl in range(2):
                    nc.sync.dma_start(
                        out=qaTl[dl][:, j * P:(j + 1) * P],
                        in_=d_qa[j * P:(j + 1) * P, dl * P:(dl + 1) * P],
                        transpose=True,
                    )
            d_qaT = dram.tile([OC, T], bf16, tag=f"d_qaT{b}",
                              name=f"d_qaT{b}")
            for dl in range(2):
                nc.sync.dma_start(out=d_qaT[dl * P:(dl + 1) * P, :],
                                  in_=qaTl[dl][:])
            d_qaTg = dram.tile([FD, T], bf16, tag=f"d_qaTg{b}",
                               name=f"d_qaTg{b}", addr_space="Shared")
            nc.gpsimd.collective_compute(
                "AllGather", ALU.bypass, replica_groups=RG,
                ins=[d_qaT.opt()], outs=[d_qaTg.opt()],
            )

        if kcut < 5:
            return
        if dbg:
            nc.sync.dma_start(out=dbg["qaT0"][:], in_=qaTl[0][:])
        # ---------------- P5: out_proj ----------------
        with (
            tc.tile_pool(name=f"yT{b}", bufs=1) as yTp,
            tc.tile_pool(name=f"rst{b}", bufs=3) as rst,
            tc.tile_pool(name=f"ynp{b}", bufs=3) as ynp,
            tc.tile_pool(name=f"ops{b}", bufs=2, space="PSUM") as ops,
            tc.tile_pool(name=f"tps2{b}", bufs=3, space="PSUM") as tps2,
        ):
            yT = [yTp.tile([P, T], fp32, tag=f"yT{o}", name=f"yT{o}")
                  for o in range(2)]
            for t4 in range(NTB):
                t4s = slice(t4 * TB, (t4 + 1) * TB)
                pso = [ops.tile([P, TB], fp32, tag=f"o{o}", name=f"o{o}")
                       for o in range(2)]
                for i in range(NF):
                    rt = rst.tile([P, TB], bf16, tag="rt", name="rt")
                    nc.sync.dma_start(out=rt[:],
                                      in_=d_qaTg[i * P:(i + 1) * P, t4s])
                    for o in range(2):
                        nc.tensor.matmul(
                            pso[o][:],
                            lhsT=wqbf["o"][i][:, o * P:(o + 1) * P],
                            rhs=rt[:],
                            start=(i == 0), stop=(i == NF - 1),
                        )
                for o in range(2):
                    nc.vector.tensor_copy(yT[o][:, t4s], pso[o][:])
            for j in range(NT):
                yn = ynp.tile([P, OC], fp32, tag="yn", name="yn")
                for o in range(2):
                    pst = tps2.tile([P, P], fp32, tag="t2", name="t2")
                    nc.tensor.transpose(pst[:], yT[o][:, j * P:(j + 1) * P],
                                        ident[:])
                    nc.vector.tensor_scalar(
                        yn[:, o * P:(o + 1) * P], pst[:],
                        Ry[:, j:j + 1], None, ALU.mult,
                    )
                nc.sync.dma_start(out=y_out[b, j * P:(j + 1) * P, :],
                                  in_=yn[:])
            if dbg:
                for i in range(NF):
                    gg = rst.tile([P, T], DT.bfloat16, tag="gg", name="gg")
                    nc.sync.dma_start(out=gg[:], in_=d_qaTg[i * P:(i + 1) * P, :])
                    nc.sync.dma_start(out=dbg["qaTg"][i * P:(i + 1) * P, :], in_=gg[:])


# ---------------------------------------------------------------------------
_CACHE = {}


def _get_nc():
    if "nc" not in _CACHE:
        _CACHE["nc"] = build_kernel()
    return _CACHE["nc"]


def prepare_in_maps(x, w_q, w_k, w_v, w_o):
    xf = np.ascontiguousarray(np.asarray(x, np.float32).reshape(BT, FD))
    ws = {"q": w_q, "k": w_k, "v": w_v, "o": w_o}
    in_maps = []
    for c in range(N_CORES):
        m = {"x": xf}
        for k, w in ws.items():
            sl = np.asarray(w, np.float32)[c * OC:(c + 1) * OC, :]
            m[f"w{k}T"] = np.ascontiguousarray(sl.T)
        in_maps.append(m)
    return in_maps


def kernel(x, w_q, w_k, w_v, w_o):
    nc = _get_nc()
    in_maps = prepare_in_maps(x, w_q, w_k, w_v, w_o)
    last_err = None
    for _attempt in range(4):
        try:
            res = run_bass_kernel_spmd(nc, in_maps, list(range(N_CORES)))
            break
        except Exception as e:  # sporadic device-unrecoverable; retry
            last_err = e
            import time as _time
            _time.sleep(2.0)
    else:
        raise last_err
    outs = [res.results[c]["y"] for c in range(N_CORES)]  # [B, T, OC] each
    y = np.concatenate(outs, axis=2)  # [B, T, FD]
    return np.ascontiguousarray(y.astype(np.float32))

